# revision 1
# baseline (speedup 1.0000x reference)
"""Trainium2 Bass kernel for nn_ConcatLayer: (N, 9) -> (N, 3).

Pure data-parallel: the batch dim is sharded across 8 NeuronCores; each core
runs an identical elementwise Bass/Tile program over its shard.

Algorithm (bit-exact reformulation of the reference, verified vs jax):
  per row v(9,) split into segments u=v[0:3], n=v[3:6], d=v[6:9]:
    m_s  = (s0 > max(s1,s2)) - (s2 > max(s0,s1))        in {-1,0,1}
    calc = m_n^2 * (m_u + m_n + m_d); sgn = clip(calc,-1,1)
    col  = 1 if calc==0 else 0 if calc==1 else 2
    z_s  = (m_s == sgn); cmp_s = z_s * v[s][col]
    row  = first argmax(cmp_u, cmp_n, cmp_d)
    out  = v[row] * z_row
All steps are elementwise over rows, so rows are laid out along the free
dimension (128 partitions x F rows each per tile) and the 9 components are
accessed through strided access-pattern views of the contiguous input tile.
"""

import numpy as np

import concourse.bass as bass
import concourse.mybir as mybir
from concourse.alu_op_type import AluOpType as A
from concourse.tile import TileContext
from concourse.bass_utils import run_bass_kernel_spmd

P = 128
N_CORES = 8
FP32 = mybir.dt.float32
BF16 = mybir.dt.bfloat16
U8 = mybir.dt.uint8
ACT = mybir.ActivationFunctionType


USE_GPSIMD = False
USE_ACT = False


def build_kernel(rows_per_core: int, f: int, reps: int = 1) -> bass.Bass:
    """Build the per-core Bass program. rows_per_core must equal 128*f*ntiles.

    reps > 1 wraps the whole (idempotent) computation in a hardware loop:
    the per-rep slope of wall time isolates HW kernel time from host-side
    dispatch/transfer overhead when benchmarking.
    """
    assert rows_per_core % (P * f) == 0
    ntiles = rows_per_core // (P * f)

    nc = bass.Bass()
    gp = nc.gpsimd if USE_GPSIMD else nc.vector
    x = nc.declare_dram_parameter("x", [rows_per_core, 9], FP32, isOutput=False)
    y = nc.declare_dram_parameter("y", [rows_per_core, 3], FP32, isOutput=True)

    from contextlib import nullcontext
    with TileContext(nc) as tc:
        with (
            tc.tile_pool(name="io", bufs=2) as io,
            tc.tile_pool(name="wk", bufs=2) as wk,
            tc.For_i(0, reps, 1) if reps > 1 else nullcontext(),
        ):
            for t in range(ntiles):
                r0 = t * P * f
                r1 = (t + 1) * P * f

                xt = io.tile([P, f * 9], FP32, tag="xt")
                nc.sync.dma_start(
                    out=xt[:],
                    in_=x[r0:r1, :].rearrange("(p f) c -> p (f c)", p=P),
                )

                # Views of the input tile.  R4[p, fi, s, c] = component c of
                # segment s of row fi.  V_c enumerates (fi, s) with s inner;
                # U/Nv/D enumerate (fi, c) with c inner.
                R4 = xt[:].rearrange("p (f s c) -> p f s c", s=3, c=3)
                V0, V1, V2 = R4[:, :, :, 0], R4[:, :, :, 1], R4[:, :, :, 2]
                U, Nv, D = R4[:, :, 0, :], R4[:, :, 1, :], R4[:, :, 2, :]

                # Blocked intermediates: (P, 3, f) = three dense (P, f) planes.
                # pfs-view re-orders to match V_c's (fi, s) enumeration.
                def pfs(tile_ap):
                    return tile_ap.rearrange("p s f -> p f s")

                mx1 = wk.tile([P, 3, f], FP32, tag="mx1")
                mx2 = wk.tile([P, 3, f], FP32, tag="mx2")
                Pt = wk.tile([P, 3, f], BF16, tag="Pt")
                Qt = wk.tile([P, 3, f], BF16, tag="Qt")
                Mt = wk.tile([P, 3, f], BF16, tag="Mt")
                Zt = wk.tile([P, 3, f], BF16, tag="Zt")
                SEL = wk.tile([P, 3, f], FP32, tag="SEL")
                CMP = wk.tile([P, 3, f], FP32, tag="CMP")
                sm = wk.tile([P, 8, f], BF16, tag="sm")  # 8 small (P,f) planes
                (t_s, t2_s, an_s, calc_s, sgn_s,
                 gun_s, gud_s, gnd_s) = (sm[:, i, :] for i in range(8))
                nbu_s = gun_s  # reuse: g_un dead after b_u
                # copy_predicated masks must be integer dtype (BIR verifier)
                msk = wk.tile([P, 4, f], mybir.dt.uint8, tag="msk")
                ceq0_s, ceq1_s, bu_s, bn_s = (msk[:, i, :] for i in range(4))
                zw = wk.tile([P, 1, f], BF16, tag="zw")
                zw_s = zw[:, 0, :]
                ot = io.tile([P, f * 3], FP32, tag="ot")
                O3 = ot[:].rearrange("p (f c) -> p f c", c=3)

                # --- segment max-index m_s = P - Q ---
                nc.vector.tensor_tensor(out=pfs(mx1[:]), in0=V1, in1=V2, op=A.max)
                nc.vector.tensor_tensor(out=pfs(mx2[:]), in0=V0, in1=V1, op=A.max)
                nc.vector.tensor_tensor(out=pfs(Pt[:]), in0=V0, in1=pfs(mx1[:]), op=A.is_gt)
                nc.vector.tensor_tensor(out=pfs(Qt[:]), in0=V2, in1=pfs(mx2[:]), op=A.is_gt)
                nc.vector.scalar_tensor_tensor(
                    out=Mt[:], in0=Qt[:], scalar=-1.0, in1=Pt[:], op0=A.mult, op1=A.add
                )
                m_u, m_n, m_d = Mt[:, 0, :], Mt[:, 1, :], Mt[:, 2, :]

                # --- calc, sgn, col masks ---
                gp.tensor_tensor(out=t_s, in0=m_u, in1=m_d, op=A.add)
                gp.tensor_tensor(out=t2_s, in0=t_s, in1=m_n, op=A.add)
                if USE_ACT:
                    nc.scalar.activation(out=an_s, in_=m_n, func=ACT.Square)
                else:
                    nc.vector.tensor_tensor(out=an_s, in0=m_n, in1=m_n, op=A.mult)
                gp.tensor_tensor(out=calc_s, in0=an_s, in1=t2_s, op=A.mult)
                nc.vector.tensor_scalar(
                    out=sgn_s, in0=calc_s, scalar1=-1.0, scalar2=1.0, op0=A.max, op1=A.min
                )
                nc.vector.tensor_scalar(
                    out=ceq0_s, in0=calc_s, scalar1=0.0, scalar2=None, op0=A.is_equal
                )
                nc.vector.tensor_scalar(
                    out=ceq1_s, in0=calc_s, scalar1=1.0, scalar2=None, op0=A.is_equal
                )

                # --- column select per segment: SEL[s] = v[s][col] ---
                (nc.scalar.copy if USE_ACT else nc.vector.tensor_copy)(out=pfs(SEL[:]), in_=V2)
                nc.vector.copy_predicated(
                    out=pfs(SEL[:]), mask=ceq1_s.broadcast_to([P, f, 3]), data=V0
                )
                nc.vector.copy_predicated(
                    out=pfs(SEL[:]), mask=ceq0_s.broadcast_to([P, f, 3]), data=V1
                )

                # --- z gates and gated comparands ---
                # (Pool TT supports arithmetic only in this walrus; compares
                # stay on DVE, the dense mult goes to Pool.)
                nc.vector.tensor_tensor(
                    out=pfs(Zt[:]), in0=pfs(Mt[:]), in1=sgn_s.broadcast_to([P, f, 3]),
                    op=A.is_equal,
                )
                gp.tensor_tensor(out=CMP[:], in0=Zt[:], in1=SEL[:], op=A.mult)
                cmp_u, cmp_n, cmp_d = CMP[:, 0, :], CMP[:, 1, :], CMP[:, 2, :]
                z_u, z_n, z_d = Zt[:, 0, :], Zt[:, 1, :], Zt[:, 2, :]

                # --- first-argmax row masks ---
                nc.vector.tensor_tensor(out=gun_s, in0=cmp_u, in1=cmp_n, op=A.is_ge)
                nc.vector.tensor_tensor(out=gud_s, in0=cmp_u, in1=cmp_d, op=A.is_ge)
                nc.vector.tensor_tensor(out=gnd_s, in0=cmp_n, in1=cmp_d, op=A.is_ge)
                nc.vector.tensor_tensor(out=bu_s, in0=gun_s, in1=gud_s, op=A.mult)
                nc.vector.tensor_scalar(
                    out=nbu_s, in0=bu_s, scalar1=-1.0, scalar2=1.0, op0=A.mult, op1=A.add
                )
                nc.vector.tensor_tensor(out=bn_s, in0=nbu_s, in1=gnd_s, op=A.mult)

                # --- winner z gate ---
                (nc.scalar.copy if USE_ACT else nc.vector.tensor_copy)(out=zw_s, in_=z_d)
                nc.vector.copy_predicated(out=zw_s, mask=bn_s, data=z_n)
                nc.vector.copy_predicated(out=zw_s, mask=bu_s, data=z_u)

                # --- output: winner segment * zw ---
                (nc.scalar.copy if USE_ACT else nc.vector.tensor_copy)(out=O3, in_=D)
                nc.vector.copy_predicated(
                    out=O3, mask=bn_s.broadcast_to([P, f, 3]), data=Nv
                )
                nc.vector.copy_predicated(
                    out=O3, mask=bu_s.broadcast_to([P, f, 3]), data=U
                )
                nc.vector.tensor_tensor(
                    out=O3, in0=O3, in1=zw_s.broadcast_to([P, f, 3]), op=A.mult
                )

                nc.sync.dma_start(
                    out=y[r0:r1, :].rearrange("(p f) c -> p (f c)", p=P),
                    in_=ot[:],
                )

    return nc


def legalize_multi_waits(nc: bass.Bass) -> None:
    """Split multi-wait sync_info into standalone EventSemaphore instructions.

    The walrus build in this environment encodes at most ONE sync-wait per
    instruction ("Too many sync wait commands" in codegen otherwise), while
    Tile emits one wait per depended-on semaphore.  Hoist all but the last
    wait onto dedicated same-engine wait instructions placed immediately
    before, which preserves per-engine program order and thus semantics.
    """
    n = 0
    for fn in nc.m.functions:
        for bb in fn.blocks:
            new_insts = []
            for inst in bb.instructions:
                si = inst.sync_info
                if si is not None and si.on_wait and len(si.on_wait) > 1:
                    waits = list(si.on_wait)
                    for w in waits[:-1]:
                        n += 1
                        new_insts.append(
                            mybir.InstEventSemaphore(
                                name=f"WSPLIT-{n}",
                                engine=inst.engine,
                                ins=[],
                                outs=[],
                                sync_info=mybir.SyncInfo(
                                    on_wait=[w], on_update=[]
                                ),
                            )
                        )
                    inst.sync_info = mybir.SyncInfo(
                        on_wait=[waits[-1]], on_update=list(si.on_update)
                    )
                new_insts.append(inst)
            bb.instructions = new_insts


def build_kernel_v2(rows_per_core: int, f: int, reps: int = 1,
                    skew: bool = True) -> bass.Bass:
    assert rows_per_core % (P * f) == 0
    ntiles = rows_per_core // (P * f)
    W = 3 * f
    Wv = W - 2

    nc = bass.Bass()
    x = nc.declare_dram_parameter("x", [rows_per_core, 9], FP32, isOutput=False)
    y = nc.declare_dram_parameter("y", [rows_per_core, 3], FP32, isOutput=True)

    from contextlib import nullcontext
    with TileContext(nc) as tc:
        with (
            tc.tile_pool(name="io", bufs=4 if skew else 2) as io,
            tc.tile_pool(name="io2", bufs=2) as io2,
            tc.tile_pool(name="wkA", bufs=2) as wkA,
            tc.tile_pool(name="wkB", bufs=2) as wkB,
            tc.For_i(0, reps, 1) if reps > 1 else nullcontext(),
        ):
            st = {}  # per-tile tiles/views carried across stages

            def dma_in(t):
                r0, r1 = t * P * f, (t + 1) * P * f
                xt = io.tile([P, f * 9], FP32, tag="xt")
                nc.sync.dma_start(
                    out=xt[:], in_=x[r0:r1, :].rearrange("(p f) c -> p (f c)", p=P)
                )
                d = {"xt": xt}
                V3 = xt[:].rearrange("p (r c) -> p r c", c=3)
                d["X0"], d["X1"], d["X2"] = V3[:, :, 0], V3[:, :, 1], V3[:, :, 2]
                S33 = xt[:].rearrange("p (k s c) -> p k s c", s=3, c=3)
                d["X0v"] = S33[:, :, :, 0]
                d["X1v"] = S33[:, :, :, 1]
                d["Uv"] = S33[:, :, 0, :]
                d["Nvv"] = S33[:, :, 1, :]
                d["Dv"] = S33[:, :, 2, :]
                st[t] = d

            def stageA(t):
                d = st[t]
                X0, X1, X2 = d["X0"], d["X1"], d["X2"]
                mx12 = wkB.tile([P, W], FP32, tag="mx12")
                mx01 = wkB.tile([P, W], FP32, tag="mx01")
                Pg = wkB.tile([P, W], BF16, tag="Pg")
                Qg = wkB.tile([P, W], BF16, tag="Qg")
                Mt = wkA.tile([P, W], BF16, tag="Mt")
                Ssum = wkB.tile([P, f], FP32, tag="Ssum")
                sq = wkB.tile([P, f], BF16, tag="sq")
                calc = wkA.tile([P, f], BF16, tag="calc")
                SEL = wkA.tile([P, W], FP32, tag="SEL")
                # DVE: the four fp32 comparisons
                nc.vector.tensor_tensor(out=mx12[:], in0=X1, in1=X2, op=A.max)
                yield
                nc.vector.tensor_tensor(out=mx01[:], in0=X0, in1=X1, op=A.max)
                yield
                nc.vector.tensor_tensor(out=Pg[:], in0=X0, in1=mx12[:], op=A.is_gt)
                # Act: column-2 base of SEL (independent of Pool chain)
                nc.scalar.copy(out=SEL[:], in_=X2)
                yield
                nc.vector.tensor_tensor(out=Qg[:], in0=X2, in1=mx01[:], op=A.is_gt)
                yield
                # Pool: m; DVE: per-row sum via reduce; Act: m_n^2
                nc.gpsimd.tensor_tensor(out=Mt[:], in0=Pg[:], in1=Qg[:], op=A.subtract)
                Mv = Mt[:].rearrange("p (k c) -> p k c", c=3)
                nc.vector.tensor_reduce(
                    out=Ssum[:], in_=Mv, axis=mybir.AxisListType.X, op=A.add
                )
                nc.scalar.activation(out=sq[:], in_=Mv[:, :, 1], func=ACT.Square)
                yield
                nc.vector.tensor_tensor(
                    out=calc[:], in0=sq[:], in1=Ssum[:], op=A.mult
                )
                d["Mt"], d["calc"], d["SEL"] = Mt, calc, SEL

            def stageB(t):
                d = st[t]
                Mt, calc, SEL = d["Mt"], d["calc"], d["SEL"]
                sgn = wkB.tile([P, f], BF16, tag="sgn")
                ceq1 = wkB.tile([P, f], U8, tag="ceq1")
                ceq0 = wkB.tile([P, f], U8, tag="ceq0")
                Zt = wkA.tile([P, W], BF16, tag="Zt")
                cmp = wkA.tile([P, W], FP32, tag="cmp")
                Mv = Mt[:].rearrange("p (k c) -> p k c", c=3)
                SELv = SEL[:].rearrange("p (k c) -> p k c", c=3)
                nc.scalar.activation(out=sgn[:], in_=calc[:], func=ACT.Sign)
                nc.vector.tensor_scalar(
                    out=ceq1[:], in0=calc[:], scalar1=1.0, scalar2=None, op0=A.is_equal
                )
                yield
                nc.vector.tensor_scalar(
                    out=ceq0[:], in0=calc[:], scalar1=0.0, scalar2=None, op0=A.is_equal
                )
                yield
                Zv = Zt[:].rearrange("p (k c) -> p k c", c=3)
                nc.vector.tensor_tensor(
                    out=Zv, in0=Mv, in1=sgn[:].broadcast_to([P, f, 3]), op=A.is_equal
                )
                yield
                nc.vector.copy_predicated(
                    out=SELv, mask=ceq1[:].broadcast_to([P, f, 3]), data=d["X0v"]
                )
                yield
                nc.vector.copy_predicated(
                    out=SELv, mask=ceq0[:].broadcast_to([P, f, 3]), data=d["X1v"]
                )
                yield
                nc.gpsimd.tensor_tensor(out=cmp[:], in0=Zt[:], in1=SEL[:], op=A.mult)
                d["Zt"], d["cmp"] = Zt, cmp

            def stageC(t):
                d = st.pop(t)
                Zt, cmp = d["Zt"], d["cmp"]
                sm = wkB.tile([P, 8, f], BF16, tag="sm")
                gun, gud, gnd, nbu, ngnd, bu, bn, bd = (
                    sm[:, i, :] for i in range(8)
                )
                wd = wkB.tile([P, f], BF16, tag="wd")
                wu8 = wkB.tile([P, f], U8, tag="wu8")
                wn8 = wkB.tile([P, f], U8, tag="wn8")
                ot = io2.tile([P, f * 3], FP32, tag="ot")
                O3v = ot[:].rearrange("p (k c) -> p k c", c=3)
                cv = cmp[:].rearrange("p (k c) -> p k c", c=3)
                c0, c1, c2 = cv[:, :, 0], cv[:, :, 1], cv[:, :, 2]
                Zv = Zt[:].rearrange("p (k c) -> p k c", c=3)
                Z0, Z1, Z2 = Zv[:, :, 0], Zv[:, :, 1], Zv[:, :, 2]

                nc.vector.tensor_tensor(out=gun, in0=c0, in1=c1, op=A.is_ge)
                yield
                nc.vector.tensor_tensor(out=gud, in0=c0, in1=c2, op=A.is_ge)
                yield
                nc.vector.tensor_tensor(out=gnd, in0=c1, in1=c2, op=A.is_ge)
                yield
                nc.gpsimd.tensor_tensor(out=bu, in0=gun, in1=gud, op=A.mult)
                nc.scalar.activation(out=nbu, in_=bu, func=ACT.Copy,
                                     scale=-1.0, bias=1.0)
                nc.scalar.activation(out=ngnd, in_=gnd, func=ACT.Copy,
                                     scale=-1.0, bias=1.0)
                nc.gpsimd.tensor_tensor(out=bn, in0=nbu, in1=gnd, op=A.mult)
                nc.gpsimd.tensor_tensor(out=bd, in0=nbu, in1=ngnd, op=A.mult)
                nc.vector.tensor_tensor(out=wu8[:], in0=bu, in1=Z0, op=A.mult)
                yield
                nc.vector.tensor_tensor(out=wn8[:], in0=bn, in1=Z1, op=A.mult)
                yield
                nc.vector.tensor_tensor(out=wd[:], in0=bd, in1=Z2, op=A.mult)
                yield
                import os
                o3eng = nc.gpsimd if os.environ.get("O3_POOL") else nc.vector
                o3eng.tensor_tensor(
                    out=O3v, in0=d["Dv"], in1=wd[:].broadcast_to([P, f, 3]), op=A.mult
                )
                yield
                nc.vector.copy_predicated(
                    out=O3v, mask=wn8[:].broadcast_to([P, f, 3]), data=d["Nvv"]
                )
                yield
                nc.vector.copy_predicated(
                    out=O3v, mask=wu8[:].broadcast_to([P, f, 3]), data=d["Uv"]
                )
                r0, r1 = t * P * f, (t + 1) * P * f
                nc.sync.dma_start(
                    out=y[r0:r1, :].rearrange("(p f) c -> p (f c)", p=P),
                    in_=ot[:],
                )

            def drain(gens):
                gens = [g for g in gens if g is not None]
                while gens:
                    nxt = []
                    for g in gens:
                        try:
                            next(g)
                            nxt.append(g)
                        except StopIteration:
                            pass
                    gens = nxt

            if skew:
                for i in range(ntiles + 3):
                    if i < ntiles:
                        dma_in(i)
                    drain([
                        stageA(i - 1) if 0 <= i - 1 < ntiles else None,
                        stageB(i - 2) if 0 <= i - 2 < ntiles else None,
                        stageC(i - 3) if 0 <= i - 3 < ntiles else None,
                    ])
            else:
                for t in range(ntiles):
                    dma_in(t)
                    drain([stageA(t)])
                    drain([stageB(t)])
                    drain([stageC(t)])

    return nc


_CACHED = {}


def _get_kernel(rows_per_core: int, f: int) -> bass.Bass:
    key = (rows_per_core, f)
    if key not in _CACHED:
        nc = build_kernel_v2(rows_per_core, f)
        nc.finalize()
        legalize_multi_waits(nc)
        _CACHED[key] = nc
    return _CACHED[key]


LAST_RES = None  # test-harness hook: BassKernelResults of the last run


def kernel(x: np.ndarray) -> np.ndarray:
    global LAST_RES
    x = np.ascontiguousarray(np.asarray(x), dtype=np.float32)
    n = x.shape[0]
    assert n % N_CORES == 0
    rpc = n // N_CORES
    f = 512
    nc = _get_kernel(rpc, f)
    shards = [x[i * rpc:(i + 1) * rpc] for i in range(N_CORES)]
    in_maps = [{"x": s} for s in shards]
    LAST_RES = run_bass_kernel_spmd(nc, in_maps, list(range(N_CORES)))
    res = LAST_RES.results
    return np.concatenate([r["y"] for r in res], axis=0)



# revision 5
# speedup vs baseline: 1.1549x; 1.1549x over previous
"""Trainium2 Bass kernel for nn_ConcatLayer: (N, 9) -> (N, 3).

Pure data-parallel: the batch dim is sharded across 8 NeuronCores; each core
runs an identical elementwise Bass/Tile program over its shard (kernel
entry point `kernel()` -> build_kernel_v4).

Algorithm (bit-exact reformulation of the reference, verified vs jax):
  per row v(9,) split into segments u=v[0:3], n=v[3:6], d=v[6:9]:
    m_s  = (s0 > max(s1,s2)) - (s2 > max(s0,s1))        in {-1,0,1}
    calc = m_n^2 * (m_u + m_n + m_d); sgn = sign(calc)
    col  = 1 if calc==0 else 0 if calc==1 else 2
    z_s  = (m_s == sgn); cmp_s = z_s * v[s][col]
    row  = first argmax(cmp_u, cmp_n, cmp_d)
    out  = v[row] * z_row

v4 engine/layout design, from HW microbenchmarks (see trn2 notes):
  - every instruction writes a DENSE tile (transposed-view writes cost ~4x
    on DVE); layout changes happen on strided/broadcast READS (cheap);
  - fp32 ordering-critical compares (mx12/mx01/Pg/Qg/ge) on DVE;
  - 3-way selects (column select, winner select) as copy_predicated with
    u8 row masks broadcast over the inner dim;
  - Pool (gpsimd, arithmetic-only) takes the cmp and output-base products;
  - Act takes square/sign/|.|-affine mask math and the SEL base copy;
  - mask algebra on packed bf16 planes (DVE 2x/4x modes are real for
    2-byte packed data; the fp32 "2x_2p" mode is not).
"""

import numpy as np
from contextlib import nullcontext

import concourse.bass as bass
import concourse.mybir as mybir
from concourse.alu_op_type import AluOpType as A
from concourse.tile import TileContext
from concourse.bass_utils import run_bass_kernel_spmd

P = 128
N_CORES = 8
FP32 = mybir.dt.float32
BF16 = mybir.dt.bfloat16
U8 = mybir.dt.uint8
ACT = mybir.ActivationFunctionType


USE_GPSIMD = False
USE_ACT = False


def build_kernel(rows_per_core: int, f: int, reps: int = 1) -> bass.Bass:
    """Build the per-core Bass program. rows_per_core must equal 128*f*ntiles.

    reps > 1 wraps the whole (idempotent) computation in a hardware loop:
    the per-rep slope of wall time isolates HW kernel time from host-side
    dispatch/transfer overhead when benchmarking.
    """
    assert rows_per_core % (P * f) == 0
    ntiles = rows_per_core // (P * f)

    nc = bass.Bass()
    gp = nc.gpsimd if USE_GPSIMD else nc.vector
    x = nc.declare_dram_parameter("x", [rows_per_core, 9], FP32, isOutput=False)
    y = nc.declare_dram_parameter("y", [rows_per_core, 3], FP32, isOutput=True)

    from contextlib import nullcontext
    with TileContext(nc) as tc:
        with (
            tc.tile_pool(name="io", bufs=2) as io,
            tc.tile_pool(name="wk", bufs=2) as wk,
            tc.For_i(0, reps, 1) if reps > 1 else nullcontext(),
        ):
            for t in range(ntiles):
                r0 = t * P * f
                r1 = (t + 1) * P * f

                xt = io.tile([P, f * 9], FP32, tag="xt")
                nc.sync.dma_start(
                    out=xt[:],
                    in_=x[r0:r1, :].rearrange("(p f) c -> p (f c)", p=P),
                )

                # Views of the input tile.  R4[p, fi, s, c] = component c of
                # segment s of row fi.  V_c enumerates (fi, s) with s inner;
                # U/Nv/D enumerate (fi, c) with c inner.
                R4 = xt[:].rearrange("p (f s c) -> p f s c", s=3, c=3)
                V0, V1, V2 = R4[:, :, :, 0], R4[:, :, :, 1], R4[:, :, :, 2]
                U, Nv, D = R4[:, :, 0, :], R4[:, :, 1, :], R4[:, :, 2, :]

                # Blocked intermediates: (P, 3, f) = three dense (P, f) planes.
                # pfs-view re-orders to match V_c's (fi, s) enumeration.
                def pfs(tile_ap):
                    return tile_ap.rearrange("p s f -> p f s")

                mx1 = wk.tile([P, 3, f], FP32, tag="mx1")
                mx2 = wk.tile([P, 3, f], FP32, tag="mx2")
                Pt = wk.tile([P, 3, f], BF16, tag="Pt")
                Qt = wk.tile([P, 3, f], BF16, tag="Qt")
                Mt = wk.tile([P, 3, f], BF16, tag="Mt")
                Zt = wk.tile([P, 3, f], BF16, tag="Zt")
                SEL = wk.tile([P, 3, f], FP32, tag="SEL")
                CMP = wk.tile([P, 3, f], FP32, tag="CMP")
                sm = wk.tile([P, 8, f], BF16, tag="sm")  # 8 small (P,f) planes
                (t_s, t2_s, an_s, calc_s, sgn_s,
                 gun_s, gud_s, gnd_s) = (sm[:, i, :] for i in range(8))
                nbu_s = gun_s  # reuse: g_un dead after b_u
                # copy_predicated masks must be integer dtype (BIR verifier)
                msk = wk.tile([P, 4, f], mybir.dt.uint8, tag="msk")
                ceq0_s, ceq1_s, bu_s, bn_s = (msk[:, i, :] for i in range(4))
                zw = wk.tile([P, 1, f], BF16, tag="zw")
                zw_s = zw[:, 0, :]
                ot = io.tile([P, f * 3], FP32, tag="ot")
                O3 = ot[:].rearrange("p (f c) -> p f c", c=3)

                # --- segment max-index m_s = P - Q ---
                nc.vector.tensor_tensor(out=pfs(mx1[:]), in0=V1, in1=V2, op=A.max)
                nc.vector.tensor_tensor(out=pfs(mx2[:]), in0=V0, in1=V1, op=A.max)
                nc.vector.tensor_tensor(out=pfs(Pt[:]), in0=V0, in1=pfs(mx1[:]), op=A.is_gt)
                nc.vector.tensor_tensor(out=pfs(Qt[:]), in0=V2, in1=pfs(mx2[:]), op=A.is_gt)
                nc.vector.scalar_tensor_tensor(
                    out=Mt[:], in0=Qt[:], scalar=-1.0, in1=Pt[:], op0=A.mult, op1=A.add
                )
                m_u, m_n, m_d = Mt[:, 0, :], Mt[:, 1, :], Mt[:, 2, :]

                # --- calc, sgn, col masks ---
                gp.tensor_tensor(out=t_s, in0=m_u, in1=m_d, op=A.add)
                gp.tensor_tensor(out=t2_s, in0=t_s, in1=m_n, op=A.add)
                if USE_ACT:
                    nc.scalar.activation(out=an_s, in_=m_n, func=ACT.Square)
                else:
                    nc.vector.tensor_tensor(out=an_s, in0=m_n, in1=m_n, op=A.mult)
                gp.tensor_tensor(out=calc_s, in0=an_s, in1=t2_s, op=A.mult)
                nc.vector.tensor_scalar(
                    out=sgn_s, in0=calc_s, scalar1=-1.0, scalar2=1.0, op0=A.max, op1=A.min
                )
                nc.vector.tensor_scalar(
                    out=ceq0_s, in0=calc_s, scalar1=0.0, scalar2=None, op0=A.is_equal
                )
                nc.vector.tensor_scalar(
                    out=ceq1_s, in0=calc_s, scalar1=1.0, scalar2=None, op0=A.is_equal
                )

                # --- column select per segment: SEL[s] = v[s][col] ---
                (nc.scalar.copy if USE_ACT else nc.vector.tensor_copy)(out=pfs(SEL[:]), in_=V2)
                nc.vector.copy_predicated(
                    out=pfs(SEL[:]), mask=ceq1_s.broadcast_to([P, f, 3]), data=V0
                )
                nc.vector.copy_predicated(
                    out=pfs(SEL[:]), mask=ceq0_s.broadcast_to([P, f, 3]), data=V1
                )

                # --- z gates and gated comparands ---
                # (Pool TT supports arithmetic only in this walrus; compares
                # stay on DVE, the dense mult goes to Pool.)
                nc.vector.tensor_tensor(
                    out=pfs(Zt[:]), in0=pfs(Mt[:]), in1=sgn_s.broadcast_to([P, f, 3]),
                    op=A.is_equal,
                )
                gp.tensor_tensor(out=CMP[:], in0=Zt[:], in1=SEL[:], op=A.mult)
                cmp_u, cmp_n, cmp_d = CMP[:, 0, :], CMP[:, 1, :], CMP[:, 2, :]
                z_u, z_n, z_d = Zt[:, 0, :], Zt[:, 1, :], Zt[:, 2, :]

                # --- first-argmax row masks ---
                nc.vector.tensor_tensor(out=gun_s, in0=cmp_u, in1=cmp_n, op=A.is_ge)
                nc.vector.tensor_tensor(out=gud_s, in0=cmp_u, in1=cmp_d, op=A.is_ge)
                nc.vector.tensor_tensor(out=gnd_s, in0=cmp_n, in1=cmp_d, op=A.is_ge)
                nc.vector.tensor_tensor(out=bu_s, in0=gun_s, in1=gud_s, op=A.mult)
                nc.vector.tensor_scalar(
                    out=nbu_s, in0=bu_s, scalar1=-1.0, scalar2=1.0, op0=A.mult, op1=A.add
                )
                nc.vector.tensor_tensor(out=bn_s, in0=nbu_s, in1=gnd_s, op=A.mult)

                # --- winner z gate ---
                (nc.scalar.copy if USE_ACT else nc.vector.tensor_copy)(out=zw_s, in_=z_d)
                nc.vector.copy_predicated(out=zw_s, mask=bn_s, data=z_n)
                nc.vector.copy_predicated(out=zw_s, mask=bu_s, data=z_u)

                # --- output: winner segment * zw ---
                (nc.scalar.copy if USE_ACT else nc.vector.tensor_copy)(out=O3, in_=D)
                nc.vector.copy_predicated(
                    out=O3, mask=bn_s.broadcast_to([P, f, 3]), data=Nv
                )
                nc.vector.copy_predicated(
                    out=O3, mask=bu_s.broadcast_to([P, f, 3]), data=U
                )
                nc.vector.tensor_tensor(
                    out=O3, in0=O3, in1=zw_s.broadcast_to([P, f, 3]), op=A.mult
                )

                nc.sync.dma_start(
                    out=y[r0:r1, :].rearrange("(p f) c -> p (f c)", p=P),
                    in_=ot[:],
                )

    return nc


def legalize_multi_waits(nc: bass.Bass) -> None:
    """Split multi-wait sync_info into standalone EventSemaphore instructions.

    The walrus build in this environment encodes at most ONE sync-wait per
    instruction ("Too many sync wait commands" in codegen otherwise), while
    Tile emits one wait per depended-on semaphore.  Hoist all but the last
    wait onto dedicated same-engine wait instructions placed immediately
    before, which preserves per-engine program order and thus semantics.
    """
    n = 0
    for fn in nc.m.functions:
        for bb in fn.blocks:
            new_insts = []
            for inst in bb.instructions:
                si = inst.sync_info
                if si is not None and si.on_wait and len(si.on_wait) > 1:
                    waits = list(si.on_wait)
                    for w in waits[:-1]:
                        n += 1
                        new_insts.append(
                            mybir.InstEventSemaphore(
                                name=f"WSPLIT-{n}",
                                engine=inst.engine,
                                ins=[],
                                outs=[],
                                sync_info=mybir.SyncInfo(
                                    on_wait=[w], on_update=[]
                                ),
                            )
                        )
                    inst.sync_info = mybir.SyncInfo(
                        on_wait=[waits[-1]], on_update=list(si.on_update)
                    )
                new_insts.append(inst)
            bb.instructions = new_insts


def build_kernel_v2(rows_per_core: int, f: int, reps: int = 1,
                    skew: bool = True) -> bass.Bass:
    assert rows_per_core % (P * f) == 0
    ntiles = rows_per_core // (P * f)
    W = 3 * f
    Wv = W - 2

    nc = bass.Bass()
    x = nc.declare_dram_parameter("x", [rows_per_core, 9], FP32, isOutput=False)
    y = nc.declare_dram_parameter("y", [rows_per_core, 3], FP32, isOutput=True)

    from contextlib import nullcontext
    with TileContext(nc) as tc:
        with (
            tc.tile_pool(name="io", bufs=4 if skew else 2) as io,
            tc.tile_pool(name="io2", bufs=2) as io2,
            tc.tile_pool(name="wkA", bufs=2) as wkA,
            tc.tile_pool(name="wkB", bufs=2) as wkB,
            tc.For_i(0, reps, 1) if reps > 1 else nullcontext(),
        ):
            st = {}  # per-tile tiles/views carried across stages

            def dma_in(t):
                r0, r1 = t * P * f, (t + 1) * P * f
                xt = io.tile([P, f * 9], FP32, tag="xt")
                nc.sync.dma_start(
                    out=xt[:], in_=x[r0:r1, :].rearrange("(p f) c -> p (f c)", p=P)
                )
                d = {"xt": xt}
                V3 = xt[:].rearrange("p (r c) -> p r c", c=3)
                d["X0"], d["X1"], d["X2"] = V3[:, :, 0], V3[:, :, 1], V3[:, :, 2]
                S33 = xt[:].rearrange("p (k s c) -> p k s c", s=3, c=3)
                d["X0v"] = S33[:, :, :, 0]
                d["X1v"] = S33[:, :, :, 1]
                d["Uv"] = S33[:, :, 0, :]
                d["Nvv"] = S33[:, :, 1, :]
                d["Dv"] = S33[:, :, 2, :]
                st[t] = d

            def stageA(t):
                d = st[t]
                X0, X1, X2 = d["X0"], d["X1"], d["X2"]
                mx12 = wkB.tile([P, W], FP32, tag="mx12")
                mx01 = wkB.tile([P, W], FP32, tag="mx01")
                Pg = wkB.tile([P, W], BF16, tag="Pg")
                Qg = wkB.tile([P, W], BF16, tag="Qg")
                Mt = wkA.tile([P, W], BF16, tag="Mt")
                Ssum = wkB.tile([P, f], FP32, tag="Ssum")
                sq = wkB.tile([P, f], BF16, tag="sq")
                calc = wkA.tile([P, f], BF16, tag="calc")
                SEL = wkA.tile([P, W], FP32, tag="SEL")
                # DVE: the four fp32 comparisons
                nc.vector.tensor_tensor(out=mx12[:], in0=X1, in1=X2, op=A.max)
                yield
                nc.vector.tensor_tensor(out=mx01[:], in0=X0, in1=X1, op=A.max)
                yield
                nc.vector.tensor_tensor(out=Pg[:], in0=X0, in1=mx12[:], op=A.is_gt)
                # Act: column-2 base of SEL (independent of Pool chain)
                nc.scalar.copy(out=SEL[:], in_=X2)
                yield
                nc.vector.tensor_tensor(out=Qg[:], in0=X2, in1=mx01[:], op=A.is_gt)
                yield
                # Pool: m; DVE: per-row sum via reduce; Act: m_n^2
                nc.gpsimd.tensor_tensor(out=Mt[:], in0=Pg[:], in1=Qg[:], op=A.subtract)
                Mv = Mt[:].rearrange("p (k c) -> p k c", c=3)
                nc.vector.tensor_reduce(
                    out=Ssum[:], in_=Mv, axis=mybir.AxisListType.X, op=A.add
                )
                nc.scalar.activation(out=sq[:], in_=Mv[:, :, 1], func=ACT.Square)
                yield
                nc.vector.tensor_tensor(
                    out=calc[:], in0=sq[:], in1=Ssum[:], op=A.mult
                )
                d["Mt"], d["calc"], d["SEL"] = Mt, calc, SEL

            def stageB(t):
                d = st[t]
                Mt, calc, SEL = d["Mt"], d["calc"], d["SEL"]
                sgn = wkB.tile([P, f], BF16, tag="sgn")
                ceq1 = wkB.tile([P, f], U8, tag="ceq1")
                ceq0 = wkB.tile([P, f], U8, tag="ceq0")
                Zt = wkA.tile([P, W], BF16, tag="Zt")
                cmp = wkA.tile([P, W], FP32, tag="cmp")
                Mv = Mt[:].rearrange("p (k c) -> p k c", c=3)
                SELv = SEL[:].rearrange("p (k c) -> p k c", c=3)
                nc.scalar.activation(out=sgn[:], in_=calc[:], func=ACT.Sign)
                nc.vector.tensor_scalar(
                    out=ceq1[:], in0=calc[:], scalar1=1.0, scalar2=None, op0=A.is_equal
                )
                yield
                nc.vector.tensor_scalar(
                    out=ceq0[:], in0=calc[:], scalar1=0.0, scalar2=None, op0=A.is_equal
                )
                yield
                Zv = Zt[:].rearrange("p (k c) -> p k c", c=3)
                nc.vector.tensor_tensor(
                    out=Zv, in0=Mv, in1=sgn[:].broadcast_to([P, f, 3]), op=A.is_equal
                )
                yield
                nc.vector.copy_predicated(
                    out=SELv, mask=ceq1[:].broadcast_to([P, f, 3]), data=d["X0v"]
                )
                yield
                nc.vector.copy_predicated(
                    out=SELv, mask=ceq0[:].broadcast_to([P, f, 3]), data=d["X1v"]
                )
                yield
                nc.gpsimd.tensor_tensor(out=cmp[:], in0=Zt[:], in1=SEL[:], op=A.mult)
                d["Zt"], d["cmp"] = Zt, cmp

            def stageC(t):
                d = st.pop(t)
                Zt, cmp = d["Zt"], d["cmp"]
                sm = wkB.tile([P, 8, f], BF16, tag="sm")
                gun, gud, gnd, nbu, ngnd, bu, bn, bd = (
                    sm[:, i, :] for i in range(8)
                )
                wd = wkB.tile([P, f], BF16, tag="wd")
                wu8 = wkB.tile([P, f], U8, tag="wu8")
                wn8 = wkB.tile([P, f], U8, tag="wn8")
                ot = io2.tile([P, f * 3], FP32, tag="ot")
                O3v = ot[:].rearrange("p (k c) -> p k c", c=3)
                cv = cmp[:].rearrange("p (k c) -> p k c", c=3)
                c0, c1, c2 = cv[:, :, 0], cv[:, :, 1], cv[:, :, 2]
                Zv = Zt[:].rearrange("p (k c) -> p k c", c=3)
                Z0, Z1, Z2 = Zv[:, :, 0], Zv[:, :, 1], Zv[:, :, 2]

                nc.vector.tensor_tensor(out=gun, in0=c0, in1=c1, op=A.is_ge)
                yield
                nc.vector.tensor_tensor(out=gud, in0=c0, in1=c2, op=A.is_ge)
                yield
                nc.vector.tensor_tensor(out=gnd, in0=c1, in1=c2, op=A.is_ge)
                yield
                nc.gpsimd.tensor_tensor(out=bu, in0=gun, in1=gud, op=A.mult)
                nc.scalar.activation(out=nbu, in_=bu, func=ACT.Copy,
                                     scale=-1.0, bias=1.0)
                nc.scalar.activation(out=ngnd, in_=gnd, func=ACT.Copy,
                                     scale=-1.0, bias=1.0)
                nc.gpsimd.tensor_tensor(out=bn, in0=nbu, in1=gnd, op=A.mult)
                nc.gpsimd.tensor_tensor(out=bd, in0=nbu, in1=ngnd, op=A.mult)
                nc.vector.tensor_tensor(out=wu8[:], in0=bu, in1=Z0, op=A.mult)
                yield
                nc.vector.tensor_tensor(out=wn8[:], in0=bn, in1=Z1, op=A.mult)
                yield
                nc.vector.tensor_tensor(out=wd[:], in0=bd, in1=Z2, op=A.mult)
                yield
                import os
                o3eng = nc.gpsimd if os.environ.get("O3_POOL") else nc.vector
                o3eng.tensor_tensor(
                    out=O3v, in0=d["Dv"], in1=wd[:].broadcast_to([P, f, 3]), op=A.mult
                )
                yield
                nc.vector.copy_predicated(
                    out=O3v, mask=wn8[:].broadcast_to([P, f, 3]), data=d["Nvv"]
                )
                yield
                nc.vector.copy_predicated(
                    out=O3v, mask=wu8[:].broadcast_to([P, f, 3]), data=d["Uv"]
                )
                r0, r1 = t * P * f, (t + 1) * P * f
                nc.sync.dma_start(
                    out=y[r0:r1, :].rearrange("(p f) c -> p (f c)", p=P),
                    in_=ot[:],
                )

            def drain(gens):
                gens = [g for g in gens if g is not None]
                while gens:
                    nxt = []
                    for g in gens:
                        try:
                            next(g)
                            nxt.append(g)
                        except StopIteration:
                            pass
                    gens = nxt

            if skew:
                for i in range(ntiles + 3):
                    if i < ntiles:
                        dma_in(i)
                    drain([
                        stageA(i - 1) if 0 <= i - 1 < ntiles else None,
                        stageB(i - 2) if 0 <= i - 2 < ntiles else None,
                        stageC(i - 3) if 0 <= i - 3 < ntiles else None,
                    ])
            else:
                for t in range(ntiles):
                    dma_in(t)
                    drain([stageA(t)])
                    drain([stageB(t)])
                    drain([stageC(t)])

    return nc


# engine-placement switches (tuned by sweep; see t_sweep.py)
OPTS = {
    "pg_arith": False,   # Pg/Qg via Pool-sub + Act-sign + DVE TS-relu
    "z_arith": False,    # z via Pool-sub + Act square/relu
    "t12_pool": False,   # t1/t2 adds on Pool
}


def build_kernel_v4(rows_per_core: int, f: int, reps: int = 1,
                    skew: bool = True) -> bass.Bass:
    assert rows_per_core % (P * f) == 0
    ntiles = rows_per_core // (P * f)

    nc = bass.Bass()
    x = nc.declare_dram_parameter("x", [rows_per_core, 9], FP32, isOutput=False)
    y = nc.declare_dram_parameter("y", [rows_per_core, 3], FP32, isOutput=True)

    with TileContext(nc) as tc:
        with (
            tc.tile_pool(name="io", bufs=3 if skew else 2) as io,
            tc.tile_pool(name="io2", bufs=2) as io2,
            tc.tile_pool(name="wkA", bufs=2) as wkA,
            tc.tile_pool(name="wkB", bufs=2) as wkB,
            tc.For_i(0, reps, 1) if reps > 1 else nullcontext(),
        ):
            st = {}

            def dma_in(t):
                r0, r1 = t * P * f, (t + 1) * P * f
                xt = io.tile([P, f * 9], FP32, tag="xt")
                nc.sync.dma_start(
                    out=xt[:], in_=x[r0:r1, :].rearrange("(p f) c -> p (f c)", p=P)
                )
                d = {"xt": xt}
                S33 = xt[:].rearrange("p (k s c) -> p k s c", s=3, c=3)
                d["X0"], d["X1"], d["X2"] = (S33[:, :, :, c] for c in range(3))
                d["U"], d["Nv"], d["D"] = (S33[:, :, s, :] for s in range(3))
                st[t] = d

            def stageA(t):
                d = st[t]
                X0, X1, X2 = d["X0"], d["X1"], d["X2"]
                mx12 = wkB.tile([P, f, 3], FP32, tag="mx12")
                mx01 = wkB.tile([P, f, 3], FP32, tag="mx01")
                Pg = wkB.tile([P, f, 3], BF16, tag="Pg")
                Qg = wkB.tile([P, f, 3], BF16, tag="Qg")
                M = wkA.tile([P, f, 3], BF16, tag="M")
                sm = wkB.tile([P, 4, f], BF16, tag="smA")
                t1_, t2_, an_, calc_ = (sm[:, i, :] for i in range(4))
                sgn = wkA.tile([P, f], BF16, tag="sgn")
                ceq0 = wkA.tile([P, f], U8, tag="ceq0")
                ceq1 = wkA.tile([P, f], U8, tag="ceq1")
                s2_ = wkB.tile([P, f], BF16, tag="s2")
                a1_ = wkB.tile([P, f], BF16, tag="a1")
                z = wkA.tile([P, 3, f], BF16, tag="z")

                nc.vector.tensor_tensor(out=mx12[:], in0=X1, in1=X2, op=A.max)
                yield
                nc.vector.tensor_tensor(out=mx01[:], in0=X0, in1=X1, op=A.max)
                yield
                if OPTS["pg_arith"]:
                    d0 = wkB.tile([P, f, 3], FP32, tag="d0")
                    d2 = wkB.tile([P, f, 3], FP32, tag="d2")
                    s0 = wkB.tile([P, f, 3], BF16, tag="s0")
                    s2g = wkB.tile([P, f, 3], BF16, tag="s2g")
                    nc.gpsimd.tensor_tensor(out=d0[:], in0=X0, in1=mx12[:],
                                            op=A.subtract)
                    yield
                    nc.gpsimd.tensor_tensor(out=d2[:], in0=X2, in1=mx01[:],
                                            op=A.subtract)
                    yield
                    nc.scalar.activation(out=s0[:], in_=d0[:], func=ACT.Sign)
                    yield
                    nc.scalar.activation(out=s2g[:], in_=d2[:], func=ACT.Sign)
                    yield
                    nc.vector.tensor_scalar(
                        out=Pg[:].rearrange("p k s -> p (k s)"),
                        in0=s0[:].rearrange("p k s -> p (k s)"),
                        scalar1=0.0, scalar2=None, op0=A.max)
                    yield
                    nc.vector.tensor_scalar(
                        out=Qg[:].rearrange("p k s -> p (k s)"),
                        in0=s2g[:].rearrange("p k s -> p (k s)"),
                        scalar1=0.0, scalar2=None, op0=A.max)
                else:
                    nc.vector.tensor_tensor(out=Pg[:], in0=X0, in1=mx12[:],
                                            op=A.is_gt)
                    yield
                    nc.vector.tensor_tensor(out=Qg[:], in0=X2, in1=mx01[:],
                                            op=A.is_gt)
                yield
                nc.vector.tensor_tensor(
                    out=M[:].rearrange("p k s -> p (k s)"),
                    in0=Pg[:].rearrange("p k s -> p (k s)"),
                    in1=Qg[:].rearrange("p k s -> p (k s)"), op=A.subtract)
                yield
                m_u, m_n, m_d = (M[:, :, s] for s in range(3))
                t12eng = nc.gpsimd if OPTS["t12_pool"] else nc.vector
                t12eng.tensor_tensor(out=t1_, in0=m_u, in1=m_d, op=A.add)
                nc.scalar.activation(out=an_, in_=m_n, func=ACT.Square)
                yield
                t12eng.tensor_tensor(out=t2_, in0=t1_, in1=m_n, op=A.add)
                yield
                nc.vector.tensor_tensor(out=calc_, in0=an_, in1=t2_, op=A.mult)
                yield
                nc.scalar.sign(out=sgn[:], in_=calc_)
                # ceq0 = 1 - sgn^2 ; ceq1 = relu(1 - |calc - 1|)   (exact {0,1})
                nc.scalar.activation(out=s2_[:], in_=sgn[:], func=ACT.Square)
                nc.scalar.activation(out=a1_[:], in_=calc_, func=ACT.Abs,
                                     scale=-1.0, bias=1.0)
                nc.scalar.activation(out=ceq0[:], in_=s2_[:], func=ACT.Identity,
                                     scale=-1.0, bias=1.0)
                nc.scalar.activation(out=ceq1[:], in_=a1_[:], func=ACT.Relu,
                                     scale=-1.0, bias=1.0)
                # z (segment-major dense write; strided/broadcast reads)
                MT = M[:].rearrange("p k s -> p s k")
                sgnB = sgn[:].broadcast_to([P, f, 3]).rearrange("p k s -> p s k")
                if OPTS["z_arith"]:
                    # z = relu(1 - (M - sgn)^2), exact on {-2..2}
                    dz = wkB.tile([P, 3, f], BF16, tag="dz")
                    zsq = wkB.tile([P, 3, f], BF16, tag="zsq")
                    nc.gpsimd.tensor_tensor(out=dz[:], in0=MT, in1=sgnB,
                                            op=A.subtract)
                    yield
                    nc.scalar.activation(out=zsq[:], in_=dz[:], func=ACT.Square)
                    yield
                    nc.scalar.activation(out=z[:], in_=zsq[:], func=ACT.Relu,
                                         scale=-1.0, bias=1.0)
                else:
                    nc.vector.tensor_tensor(out=z[:], in0=MT, in1=sgnB,
                                            op=A.is_equal)
                d["z"], d["ceq0"], d["ceq1"] = z, ceq0, ceq1

            def stageB(t):
                d = st[t]
                z, ceq0, ceq1 = d["z"], d["ceq0"], d["ceq1"]
                SEL = wkA.tile([P, f, 3], FP32, tag="SEL")
                cmp = wkA.tile([P, 3, f], FP32, tag="cmp")
                sm = wkB.tile([P, 8, f], BF16, tag="smB")
                gun, gud, gnd, bu, bn, nbu, ngnd, bd = (
                    sm[:, i, :] for i in range(8)
                )
                wd = wkA.tile([P, f], BF16, tag="wd")
                wu8 = wkA.tile([P, f], U8, tag="wu8")
                wn8 = wkA.tile([P, f], U8, tag="wn8")

                nc.scalar.copy(out=SEL[:], in_=d["X2"])
                yield
                nc.vector.copy_predicated(
                    out=SEL[:], mask=ceq0[:].broadcast_to([P, f, 3]), data=d["X1"]
                )
                yield
                nc.vector.copy_predicated(
                    out=SEL[:], mask=ceq1[:].broadcast_to([P, f, 3]), data=d["X0"]
                )
                yield
                nc.gpsimd.tensor_tensor(
                    out=cmp[:], in0=SEL[:].rearrange("p k s -> p s k"), in1=z[:],
                    op=A.mult)
                yield
                c_u, c_n, c_d = (cmp[:, s, :] for s in range(3))
                nc.vector.tensor_tensor(out=gun, in0=c_u, in1=c_n, op=A.is_ge)
                yield
                nc.vector.tensor_tensor(out=gud, in0=c_u, in1=c_d, op=A.is_ge)
                yield
                nc.vector.tensor_tensor(out=gnd, in0=c_n, in1=c_d, op=A.is_ge)
                yield
                nc.vector.tensor_tensor(out=bu, in0=gun, in1=gud, op=A.mult)
                yield
                nc.vector.tensor_tensor(out=bn, in0=gnd, in1=bu, op=A.is_gt)
                nc.vector.tensor_scalar(out=nbu, in0=bu, scalar1=-1.0,
                                        scalar2=1.0, op0=A.mult, op1=A.add)
                nc.vector.tensor_scalar(out=ngnd, in0=gnd, scalar1=-1.0,
                                        scalar2=1.0, op0=A.mult, op1=A.add)
                yield
                nc.vector.tensor_tensor(out=bd, in0=nbu, in1=ngnd, op=A.mult)
                z_u, z_n, z_d = (z[:, s, :] for s in range(3))
                nc.vector.tensor_tensor(out=wu8[:], in0=bu, in1=z_u, op=A.mult)
                yield
                nc.vector.tensor_tensor(out=wn8[:], in0=bn, in1=z_n, op=A.mult)
                yield
                nc.vector.tensor_tensor(out=wd[:], in0=bd, in1=z_d, op=A.mult)
                d["wd"], d["wu8"], d["wn8"] = wd, wu8, wn8

            def stageC(t):
                d = st.pop(t)
                ot = io2.tile([P, f * 3], FP32, tag="ot")
                O3 = ot[:].rearrange("p (k c) -> p k c", c=3)
                nc.gpsimd.tensor_tensor(
                    out=O3, in0=d["D"], in1=d["wd"][:].broadcast_to([P, f, 3]),
                    op=A.mult,
                )
                yield
                nc.vector.copy_predicated(
                    out=O3, mask=d["wn8"][:].broadcast_to([P, f, 3]), data=d["Nv"]
                )
                yield
                nc.vector.copy_predicated(
                    out=O3, mask=d["wu8"][:].broadcast_to([P, f, 3]), data=d["U"]
                )
                r0, r1 = t * P * f, (t + 1) * P * f
                nc.sync.dma_start(
                    out=y[r0:r1, :].rearrange("(p f) c -> p (f c)", p=P),
                    in_=ot[:],
                )

            def drain(gens):
                gens = [g for g in gens if g is not None]
                while gens:
                    nxt = []
                    for g in gens:
                        try:
                            next(g)
                            nxt.append(g)
                        except StopIteration:
                            pass
                    gens = nxt

            def chain(t):
                yield from stageB(t)
                yield from stageC(t)

            if skew:
                for i in range(ntiles + 2):
                    if i < ntiles:
                        dma_in(i)
                    drain([
                        stageA(i - 1) if 0 <= i - 1 < ntiles else None,
                        chain(i - 2) if 0 <= i - 2 < ntiles else None,
                    ])
            else:
                for t in range(ntiles):
                    dma_in(t)
                    drain([stageA(t)])
                    drain([stageB(t)])
                    drain([stageC(t)])

    return nc


_CACHED = {}


def _get_kernel(rows_per_core: int, f: int) -> bass.Bass:
    key = (rows_per_core, f)
    if key not in _CACHED:
        nc = build_kernel_v4(rows_per_core, f)
        nc.finalize()
        legalize_multi_waits(nc)
        _CACHED[key] = nc
    return _CACHED[key]


LAST_RES = None  # test-harness hook: BassKernelResults of the last run


def kernel(x: np.ndarray) -> np.ndarray:
    global LAST_RES
    x = np.ascontiguousarray(np.asarray(x), dtype=np.float32)
    n = x.shape[0]
    assert n % N_CORES == 0
    rpc = n // N_CORES
    f = 512
    nc = _get_kernel(rpc, f)
    shards = [x[i * rpc:(i + 1) * rpc] for i in range(N_CORES)]
    in_maps = [{"x": s} for s in shards]
    LAST_RES = run_bass_kernel_spmd(nc, in_maps, list(range(N_CORES)))
    res = LAST_RES.results
    return np.concatenate([r["y"] for r in res], axis=0)



# revision 6
# speedup vs baseline: 1.2194x; 1.0558x over previous
"""Trainium2 Bass kernel for nn_ConcatLayer: (N, 9) -> (N, 3).

Pure data-parallel: the batch dim is sharded across 8 NeuronCores; each core
runs an identical elementwise Bass/Tile program over its shard (kernel
entry point `kernel()` -> build_kernel_v4).

Algorithm (bit-exact reformulation of the reference, verified vs jax):
  per row v(9,) split into segments u=v[0:3], n=v[3:6], d=v[6:9]:
    m_s  = (s0 > max(s1,s2)) - (s2 > max(s0,s1))        in {-1,0,1}
    calc = m_n^2 * (m_u + m_n + m_d); sgn = sign(calc)
    col  = 1 if calc==0 else 0 if calc==1 else 2
    z_s  = (m_s == sgn); cmp_s = z_s * v[s][col]
    row  = first argmax(cmp_u, cmp_n, cmp_d)
    out  = v[row] * z_row

v4 engine/layout design, from HW microbenchmarks (see trn2 notes):
  - every instruction writes a DENSE tile (transposed-view writes cost ~4x
    on DVE); layout changes happen on strided/broadcast READS (cheap);
  - fp32 ordering-critical compares (mx12/mx01/Pg/Qg/ge) on DVE;
  - 3-way selects (column select, winner select) as copy_predicated with
    u8 row masks broadcast over the inner dim;
  - Pool (gpsimd, arithmetic-only) takes the cmp and output-base products;
  - Act takes square/sign/|.|-affine mask math and the SEL base copy;
  - mask algebra on packed bf16 planes (DVE 2x/4x modes are real for
    2-byte packed data; the fp32 "2x_2p" mode is not).
"""

import numpy as np
from contextlib import nullcontext

import concourse.bass as bass
import concourse.mybir as mybir
from concourse.alu_op_type import AluOpType as A
from concourse.tile import TileContext
from concourse.bass_utils import run_bass_kernel_spmd

P = 128
N_CORES = 8
FP32 = mybir.dt.float32
BF16 = mybir.dt.bfloat16
U8 = mybir.dt.uint8
ACT = mybir.ActivationFunctionType


USE_GPSIMD = False
USE_ACT = False


def build_kernel(rows_per_core: int, f: int, reps: int = 1) -> bass.Bass:
    """Build the per-core Bass program. rows_per_core must equal 128*f*ntiles.

    reps > 1 wraps the whole (idempotent) computation in a hardware loop:
    the per-rep slope of wall time isolates HW kernel time from host-side
    dispatch/transfer overhead when benchmarking.
    """
    assert rows_per_core % (P * f) == 0
    ntiles = rows_per_core // (P * f)

    nc = bass.Bass()
    gp = nc.gpsimd if USE_GPSIMD else nc.vector
    x = nc.declare_dram_parameter("x", [rows_per_core, 9], FP32, isOutput=False)
    y = nc.declare_dram_parameter("y", [rows_per_core, 3], FP32, isOutput=True)

    from contextlib import nullcontext
    with TileContext(nc) as tc:
        with (
            tc.tile_pool(name="io", bufs=2) as io,
            tc.tile_pool(name="wk", bufs=2) as wk,
            tc.For_i(0, reps, 1) if reps > 1 else nullcontext(),
        ):
            for t in range(ntiles):
                r0 = t * P * f
                r1 = (t + 1) * P * f

                xt = io.tile([P, f * 9], FP32, tag="xt")
                nc.sync.dma_start(
                    out=xt[:],
                    in_=x[r0:r1, :].rearrange("(p f) c -> p (f c)", p=P),
                )

                # Views of the input tile.  R4[p, fi, s, c] = component c of
                # segment s of row fi.  V_c enumerates (fi, s) with s inner;
                # U/Nv/D enumerate (fi, c) with c inner.
                R4 = xt[:].rearrange("p (f s c) -> p f s c", s=3, c=3)
                V0, V1, V2 = R4[:, :, :, 0], R4[:, :, :, 1], R4[:, :, :, 2]
                U, Nv, D = R4[:, :, 0, :], R4[:, :, 1, :], R4[:, :, 2, :]

                # Blocked intermediates: (P, 3, f) = three dense (P, f) planes.
                # pfs-view re-orders to match V_c's (fi, s) enumeration.
                def pfs(tile_ap):
                    return tile_ap.rearrange("p s f -> p f s")

                mx1 = wk.tile([P, 3, f], FP32, tag="mx1")
                mx2 = wk.tile([P, 3, f], FP32, tag="mx2")
                Pt = wk.tile([P, 3, f], BF16, tag="Pt")
                Qt = wk.tile([P, 3, f], BF16, tag="Qt")
                Mt = wk.tile([P, 3, f], BF16, tag="Mt")
                Zt = wk.tile([P, 3, f], BF16, tag="Zt")
                SEL = wk.tile([P, 3, f], FP32, tag="SEL")
                CMP = wk.tile([P, 3, f], FP32, tag="CMP")
                sm = wk.tile([P, 8, f], BF16, tag="sm")  # 8 small (P,f) planes
                (t_s, t2_s, an_s, calc_s, sgn_s,
                 gun_s, gud_s, gnd_s) = (sm[:, i, :] for i in range(8))
                nbu_s = gun_s  # reuse: g_un dead after b_u
                # copy_predicated masks must be integer dtype (BIR verifier)
                msk = wk.tile([P, 4, f], mybir.dt.uint8, tag="msk")
                ceq0_s, ceq1_s, bu_s, bn_s = (msk[:, i, :] for i in range(4))
                zw = wk.tile([P, 1, f], BF16, tag="zw")
                zw_s = zw[:, 0, :]
                ot = io.tile([P, f * 3], FP32, tag="ot")
                O3 = ot[:].rearrange("p (f c) -> p f c", c=3)

                # --- segment max-index m_s = P - Q ---
                nc.vector.tensor_tensor(out=pfs(mx1[:]), in0=V1, in1=V2, op=A.max)
                nc.vector.tensor_tensor(out=pfs(mx2[:]), in0=V0, in1=V1, op=A.max)
                nc.vector.tensor_tensor(out=pfs(Pt[:]), in0=V0, in1=pfs(mx1[:]), op=A.is_gt)
                nc.vector.tensor_tensor(out=pfs(Qt[:]), in0=V2, in1=pfs(mx2[:]), op=A.is_gt)
                nc.vector.scalar_tensor_tensor(
                    out=Mt[:], in0=Qt[:], scalar=-1.0, in1=Pt[:], op0=A.mult, op1=A.add
                )
                m_u, m_n, m_d = Mt[:, 0, :], Mt[:, 1, :], Mt[:, 2, :]

                # --- calc, sgn, col masks ---
                gp.tensor_tensor(out=t_s, in0=m_u, in1=m_d, op=A.add)
                gp.tensor_tensor(out=t2_s, in0=t_s, in1=m_n, op=A.add)
                if USE_ACT:
                    nc.scalar.activation(out=an_s, in_=m_n, func=ACT.Square)
                else:
                    nc.vector.tensor_tensor(out=an_s, in0=m_n, in1=m_n, op=A.mult)
                gp.tensor_tensor(out=calc_s, in0=an_s, in1=t2_s, op=A.mult)
                nc.vector.tensor_scalar(
                    out=sgn_s, in0=calc_s, scalar1=-1.0, scalar2=1.0, op0=A.max, op1=A.min
                )
                nc.vector.tensor_scalar(
                    out=ceq0_s, in0=calc_s, scalar1=0.0, scalar2=None, op0=A.is_equal
                )
                nc.vector.tensor_scalar(
                    out=ceq1_s, in0=calc_s, scalar1=1.0, scalar2=None, op0=A.is_equal
                )

                # --- column select per segment: SEL[s] = v[s][col] ---
                (nc.scalar.copy if USE_ACT else nc.vector.tensor_copy)(out=pfs(SEL[:]), in_=V2)
                nc.vector.copy_predicated(
                    out=pfs(SEL[:]), mask=ceq1_s.broadcast_to([P, f, 3]), data=V0
                )
                nc.vector.copy_predicated(
                    out=pfs(SEL[:]), mask=ceq0_s.broadcast_to([P, f, 3]), data=V1
                )

                # --- z gates and gated comparands ---
                # (Pool TT supports arithmetic only in this walrus; compares
                # stay on DVE, the dense mult goes to Pool.)
                nc.vector.tensor_tensor(
                    out=pfs(Zt[:]), in0=pfs(Mt[:]), in1=sgn_s.broadcast_to([P, f, 3]),
                    op=A.is_equal,
                )
                gp.tensor_tensor(out=CMP[:], in0=Zt[:], in1=SEL[:], op=A.mult)
                cmp_u, cmp_n, cmp_d = CMP[:, 0, :], CMP[:, 1, :], CMP[:, 2, :]
                z_u, z_n, z_d = Zt[:, 0, :], Zt[:, 1, :], Zt[:, 2, :]

                # --- first-argmax row masks ---
                nc.vector.tensor_tensor(out=gun_s, in0=cmp_u, in1=cmp_n, op=A.is_ge)
                nc.vector.tensor_tensor(out=gud_s, in0=cmp_u, in1=cmp_d, op=A.is_ge)
                nc.vector.tensor_tensor(out=gnd_s, in0=cmp_n, in1=cmp_d, op=A.is_ge)
                nc.vector.tensor_tensor(out=bu_s, in0=gun_s, in1=gud_s, op=A.mult)
                nc.vector.tensor_scalar(
                    out=nbu_s, in0=bu_s, scalar1=-1.0, scalar2=1.0, op0=A.mult, op1=A.add
                )
                nc.vector.tensor_tensor(out=bn_s, in0=nbu_s, in1=gnd_s, op=A.mult)

                # --- winner z gate ---
                (nc.scalar.copy if USE_ACT else nc.vector.tensor_copy)(out=zw_s, in_=z_d)
                nc.vector.copy_predicated(out=zw_s, mask=bn_s, data=z_n)
                nc.vector.copy_predicated(out=zw_s, mask=bu_s, data=z_u)

                # --- output: winner segment * zw ---
                (nc.scalar.copy if USE_ACT else nc.vector.tensor_copy)(out=O3, in_=D)
                nc.vector.copy_predicated(
                    out=O3, mask=bn_s.broadcast_to([P, f, 3]), data=Nv
                )
                nc.vector.copy_predicated(
                    out=O3, mask=bu_s.broadcast_to([P, f, 3]), data=U
                )
                nc.vector.tensor_tensor(
                    out=O3, in0=O3, in1=zw_s.broadcast_to([P, f, 3]), op=A.mult
                )

                nc.sync.dma_start(
                    out=y[r0:r1, :].rearrange("(p f) c -> p (f c)", p=P),
                    in_=ot[:],
                )

    return nc


def legalize_multi_waits(nc: bass.Bass) -> None:
    """Split multi-wait sync_info into standalone EventSemaphore instructions.

    The walrus build in this environment encodes at most ONE sync-wait per
    instruction ("Too many sync wait commands" in codegen otherwise), while
    Tile emits one wait per depended-on semaphore.  Hoist all but the last
    wait onto dedicated same-engine wait instructions placed immediately
    before, which preserves per-engine program order and thus semantics.
    """
    n = 0
    for fn in nc.m.functions:
        for bb in fn.blocks:
            new_insts = []
            for inst in bb.instructions:
                si = inst.sync_info
                if si is not None and si.on_wait and len(si.on_wait) > 1:
                    waits = list(si.on_wait)
                    for w in waits[:-1]:
                        n += 1
                        new_insts.append(
                            mybir.InstEventSemaphore(
                                name=f"WSPLIT-{n}",
                                engine=inst.engine,
                                ins=[],
                                outs=[],
                                sync_info=mybir.SyncInfo(
                                    on_wait=[w], on_update=[]
                                ),
                            )
                        )
                    inst.sync_info = mybir.SyncInfo(
                        on_wait=[waits[-1]], on_update=list(si.on_update)
                    )
                new_insts.append(inst)
            bb.instructions = new_insts


def build_kernel_v2(rows_per_core: int, f: int, reps: int = 1,
                    skew: bool = True) -> bass.Bass:
    assert rows_per_core % (P * f) == 0
    ntiles = rows_per_core // (P * f)
    W = 3 * f
    Wv = W - 2

    nc = bass.Bass()
    x = nc.declare_dram_parameter("x", [rows_per_core, 9], FP32, isOutput=False)
    y = nc.declare_dram_parameter("y", [rows_per_core, 3], FP32, isOutput=True)

    from contextlib import nullcontext
    with TileContext(nc) as tc:
        with (
            tc.tile_pool(name="io", bufs=4 if skew else 2) as io,
            tc.tile_pool(name="io2", bufs=2) as io2,
            tc.tile_pool(name="wkA", bufs=2) as wkA,
            tc.tile_pool(name="wkB", bufs=2) as wkB,
            tc.For_i(0, reps, 1) if reps > 1 else nullcontext(),
        ):
            st = {}  # per-tile tiles/views carried across stages

            def dma_in(t):
                r0, r1 = t * P * f, (t + 1) * P * f
                xt = io.tile([P, f * 9], FP32, tag="xt")
                nc.sync.dma_start(
                    out=xt[:], in_=x[r0:r1, :].rearrange("(p f) c -> p (f c)", p=P)
                )
                d = {"xt": xt}
                V3 = xt[:].rearrange("p (r c) -> p r c", c=3)
                d["X0"], d["X1"], d["X2"] = V3[:, :, 0], V3[:, :, 1], V3[:, :, 2]
                S33 = xt[:].rearrange("p (k s c) -> p k s c", s=3, c=3)
                d["X0v"] = S33[:, :, :, 0]
                d["X1v"] = S33[:, :, :, 1]
                d["Uv"] = S33[:, :, 0, :]
                d["Nvv"] = S33[:, :, 1, :]
                d["Dv"] = S33[:, :, 2, :]
                st[t] = d

            def stageA(t):
                d = st[t]
                X0, X1, X2 = d["X0"], d["X1"], d["X2"]
                mx12 = wkB.tile([P, W], FP32, tag="mx12")
                mx01 = wkB.tile([P, W], FP32, tag="mx01")
                Pg = wkB.tile([P, W], BF16, tag="Pg")
                Qg = wkB.tile([P, W], BF16, tag="Qg")
                Mt = wkA.tile([P, W], BF16, tag="Mt")
                Ssum = wkB.tile([P, f], FP32, tag="Ssum")
                sq = wkB.tile([P, f], BF16, tag="sq")
                calc = wkA.tile([P, f], BF16, tag="calc")
                SEL = wkA.tile([P, W], FP32, tag="SEL")
                # DVE: the four fp32 comparisons
                nc.vector.tensor_tensor(out=mx12[:], in0=X1, in1=X2, op=A.max)
                yield
                nc.vector.tensor_tensor(out=mx01[:], in0=X0, in1=X1, op=A.max)
                yield
                nc.vector.tensor_tensor(out=Pg[:], in0=X0, in1=mx12[:], op=A.is_gt)
                # Act: column-2 base of SEL (independent of Pool chain)
                nc.scalar.copy(out=SEL[:], in_=X2)
                yield
                nc.vector.tensor_tensor(out=Qg[:], in0=X2, in1=mx01[:], op=A.is_gt)
                yield
                # Pool: m; DVE: per-row sum via reduce; Act: m_n^2
                nc.gpsimd.tensor_tensor(out=Mt[:], in0=Pg[:], in1=Qg[:], op=A.subtract)
                Mv = Mt[:].rearrange("p (k c) -> p k c", c=3)
                nc.vector.tensor_reduce(
                    out=Ssum[:], in_=Mv, axis=mybir.AxisListType.X, op=A.add
                )
                nc.scalar.activation(out=sq[:], in_=Mv[:, :, 1], func=ACT.Square)
                yield
                nc.vector.tensor_tensor(
                    out=calc[:], in0=sq[:], in1=Ssum[:], op=A.mult
                )
                d["Mt"], d["calc"], d["SEL"] = Mt, calc, SEL

            def stageB(t):
                d = st[t]
                Mt, calc, SEL = d["Mt"], d["calc"], d["SEL"]
                sgn = wkB.tile([P, f], BF16, tag="sgn")
                ceq1 = wkB.tile([P, f], U8, tag="ceq1")
                ceq0 = wkB.tile([P, f], U8, tag="ceq0")
                Zt = wkA.tile([P, W], BF16, tag="Zt")
                cmp = wkA.tile([P, W], FP32, tag="cmp")
                Mv = Mt[:].rearrange("p (k c) -> p k c", c=3)
                SELv = SEL[:].rearrange("p (k c) -> p k c", c=3)
                nc.scalar.activation(out=sgn[:], in_=calc[:], func=ACT.Sign)
                nc.vector.tensor_scalar(
                    out=ceq1[:], in0=calc[:], scalar1=1.0, scalar2=None, op0=A.is_equal
                )
                yield
                nc.vector.tensor_scalar(
                    out=ceq0[:], in0=calc[:], scalar1=0.0, scalar2=None, op0=A.is_equal
                )
                yield
                Zv = Zt[:].rearrange("p (k c) -> p k c", c=3)
                nc.vector.tensor_tensor(
                    out=Zv, in0=Mv, in1=sgn[:].broadcast_to([P, f, 3]), op=A.is_equal
                )
                yield
                nc.vector.copy_predicated(
                    out=SELv, mask=ceq1[:].broadcast_to([P, f, 3]), data=d["X0v"]
                )
                yield
                nc.vector.copy_predicated(
                    out=SELv, mask=ceq0[:].broadcast_to([P, f, 3]), data=d["X1v"]
                )
                yield
                nc.gpsimd.tensor_tensor(out=cmp[:], in0=Zt[:], in1=SEL[:], op=A.mult)
                d["Zt"], d["cmp"] = Zt, cmp

            def stageC(t):
                d = st.pop(t)
                Zt, cmp = d["Zt"], d["cmp"]
                sm = wkB.tile([P, 8, f], BF16, tag="sm")
                gun, gud, gnd, nbu, ngnd, bu, bn, bd = (
                    sm[:, i, :] for i in range(8)
                )
                wd = wkB.tile([P, f], BF16, tag="wd")
                wu8 = wkB.tile([P, f], U8, tag="wu8")
                wn8 = wkB.tile([P, f], U8, tag="wn8")
                ot = io2.tile([P, f * 3], FP32, tag="ot")
                O3v = ot[:].rearrange("p (k c) -> p k c", c=3)
                cv = cmp[:].rearrange("p (k c) -> p k c", c=3)
                c0, c1, c2 = cv[:, :, 0], cv[:, :, 1], cv[:, :, 2]
                Zv = Zt[:].rearrange("p (k c) -> p k c", c=3)
                Z0, Z1, Z2 = Zv[:, :, 0], Zv[:, :, 1], Zv[:, :, 2]

                nc.vector.tensor_tensor(out=gun, in0=c0, in1=c1, op=A.is_ge)
                yield
                nc.vector.tensor_tensor(out=gud, in0=c0, in1=c2, op=A.is_ge)
                yield
                nc.vector.tensor_tensor(out=gnd, in0=c1, in1=c2, op=A.is_ge)
                yield
                nc.gpsimd.tensor_tensor(out=bu, in0=gun, in1=gud, op=A.mult)
                nc.scalar.activation(out=nbu, in_=bu, func=ACT.Copy,
                                     scale=-1.0, bias=1.0)
                nc.scalar.activation(out=ngnd, in_=gnd, func=ACT.Copy,
                                     scale=-1.0, bias=1.0)
                nc.gpsimd.tensor_tensor(out=bn, in0=nbu, in1=gnd, op=A.mult)
                nc.gpsimd.tensor_tensor(out=bd, in0=nbu, in1=ngnd, op=A.mult)
                nc.vector.tensor_tensor(out=wu8[:], in0=bu, in1=Z0, op=A.mult)
                yield
                nc.vector.tensor_tensor(out=wn8[:], in0=bn, in1=Z1, op=A.mult)
                yield
                nc.vector.tensor_tensor(out=wd[:], in0=bd, in1=Z2, op=A.mult)
                yield
                import os
                o3eng = nc.gpsimd if os.environ.get("O3_POOL") else nc.vector
                o3eng.tensor_tensor(
                    out=O3v, in0=d["Dv"], in1=wd[:].broadcast_to([P, f, 3]), op=A.mult
                )
                yield
                nc.vector.copy_predicated(
                    out=O3v, mask=wn8[:].broadcast_to([P, f, 3]), data=d["Nvv"]
                )
                yield
                nc.vector.copy_predicated(
                    out=O3v, mask=wu8[:].broadcast_to([P, f, 3]), data=d["Uv"]
                )
                r0, r1 = t * P * f, (t + 1) * P * f
                nc.sync.dma_start(
                    out=y[r0:r1, :].rearrange("(p f) c -> p (f c)", p=P),
                    in_=ot[:],
                )

            def drain(gens):
                gens = [g for g in gens if g is not None]
                while gens:
                    nxt = []
                    for g in gens:
                        try:
                            next(g)
                            nxt.append(g)
                        except StopIteration:
                            pass
                    gens = nxt

            if skew:
                for i in range(ntiles + 3):
                    if i < ntiles:
                        dma_in(i)
                    drain([
                        stageA(i - 1) if 0 <= i - 1 < ntiles else None,
                        stageB(i - 2) if 0 <= i - 2 < ntiles else None,
                        stageC(i - 3) if 0 <= i - 3 < ntiles else None,
                    ])
            else:
                for t in range(ntiles):
                    dma_in(t)
                    drain([stageA(t)])
                    drain([stageB(t)])
                    drain([stageC(t)])

    return nc


# engine-placement switches (tuned by sweep; see t_sweep.py)
OPTS = {
    "pg_arith": False,   # Pg/Qg via Pool-sub + Act-sign + DVE TS-relu
    "z_arith": False,    # z via Pool-sub + Act square/relu
    "t12_pool": False,   # t1/t2 adds on Pool
}


def build_kernel_v4(rows_per_core: int, f: int, reps: int = 1,
                    skew: bool = True) -> bass.Bass:
    assert rows_per_core % (P * f) == 0
    ntiles = rows_per_core // (P * f)

    nc = bass.Bass()
    x = nc.declare_dram_parameter("x", [rows_per_core, 9], FP32, isOutput=False)
    y = nc.declare_dram_parameter("y", [rows_per_core, 3], FP32, isOutput=True)

    with TileContext(nc) as tc:
        with (
            tc.tile_pool(name="io", bufs=3 if skew else 2) as io,
            tc.tile_pool(name="io2", bufs=2) as io2,
            tc.tile_pool(name="wkA", bufs=2) as wkA,
            tc.tile_pool(name="wkB", bufs=2) as wkB,
            tc.For_i(0, reps, 1) if reps > 1 else nullcontext(),
        ):
            st = {}

            def dma_in(t):
                r0, r1 = t * P * f, (t + 1) * P * f
                xt = io.tile([P, f * 9], FP32, tag="xt")
                nc.sync.dma_start(
                    out=xt[:], in_=x[r0:r1, :].rearrange("(p f) c -> p (f c)", p=P)
                )
                d = {"xt": xt}
                S33 = xt[:].rearrange("p (k s c) -> p k s c", s=3, c=3)
                d["X0"], d["X1"], d["X2"] = (S33[:, :, :, c] for c in range(3))
                d["U"], d["Nv"], d["D"] = (S33[:, :, s, :] for s in range(3))
                st[t] = d

            def stageA(t):
                d = st[t]
                X0, X1, X2 = d["X0"], d["X1"], d["X2"]
                mx12 = wkB.tile([P, f, 3], FP32, tag="mx12")
                mx01 = wkB.tile([P, f, 3], FP32, tag="mx01")
                Pg = wkB.tile([P, f, 3], BF16, tag="Pg")
                Qg = wkB.tile([P, f, 3], BF16, tag="Qg")
                M = wkA.tile([P, f, 3], BF16, tag="M")
                sm = wkB.tile([P, 4, f], BF16, tag="smA")
                t1_, t2_, an_, calc_ = (sm[:, i, :] for i in range(4))
                sgn = wkA.tile([P, f], BF16, tag="sgn")
                ceq0 = wkA.tile([P, f], U8, tag="ceq0")
                ceq1 = wkA.tile([P, f], U8, tag="ceq1")
                s2_ = wkB.tile([P, f], BF16, tag="s2")
                a1_ = wkB.tile([P, f], BF16, tag="a1")
                z = wkA.tile([P, 3, f], BF16, tag="z")

                nc.vector.tensor_tensor(out=mx12[:], in0=X1, in1=X2, op=A.max)
                yield
                nc.vector.tensor_tensor(out=mx01[:], in0=X0, in1=X1, op=A.max)
                yield
                if OPTS["pg_arith"]:
                    d0 = wkB.tile([P, f, 3], FP32, tag="d0")
                    d2 = wkB.tile([P, f, 3], FP32, tag="d2")
                    s0 = wkB.tile([P, f, 3], BF16, tag="s0")
                    s2g = wkB.tile([P, f, 3], BF16, tag="s2g")
                    nc.gpsimd.tensor_tensor(out=d0[:], in0=X0, in1=mx12[:],
                                            op=A.subtract)
                    yield
                    nc.gpsimd.tensor_tensor(out=d2[:], in0=X2, in1=mx01[:],
                                            op=A.subtract)
                    yield
                    nc.scalar.activation(out=s0[:], in_=d0[:], func=ACT.Sign)
                    yield
                    nc.scalar.activation(out=s2g[:], in_=d2[:], func=ACT.Sign)
                    yield
                    nc.vector.tensor_scalar(
                        out=Pg[:].rearrange("p k s -> p (k s)"),
                        in0=s0[:].rearrange("p k s -> p (k s)"),
                        scalar1=0.0, scalar2=None, op0=A.max)
                    yield
                    nc.vector.tensor_scalar(
                        out=Qg[:].rearrange("p k s -> p (k s)"),
                        in0=s2g[:].rearrange("p k s -> p (k s)"),
                        scalar1=0.0, scalar2=None, op0=A.max)
                else:
                    nc.vector.tensor_tensor(out=Pg[:], in0=X0, in1=mx12[:],
                                            op=A.is_gt)
                    yield
                    nc.vector.tensor_tensor(out=Qg[:], in0=X2, in1=mx01[:],
                                            op=A.is_gt)
                yield
                nc.vector.tensor_tensor(
                    out=M[:].rearrange("p k s -> p (k s)"),
                    in0=Pg[:].rearrange("p k s -> p (k s)"),
                    in1=Qg[:].rearrange("p k s -> p (k s)"), op=A.subtract)
                yield
                m_u, m_n, m_d = (M[:, :, s] for s in range(3))
                t12eng = nc.gpsimd if OPTS["t12_pool"] else nc.vector
                t12eng.tensor_tensor(out=t1_, in0=m_u, in1=m_d, op=A.add)
                nc.scalar.activation(out=an_, in_=m_n, func=ACT.Square)
                yield
                t12eng.tensor_tensor(out=t2_, in0=t1_, in1=m_n, op=A.add)
                yield
                nc.vector.tensor_tensor(out=calc_, in0=an_, in1=t2_, op=A.mult)
                yield
                nc.scalar.sign(out=sgn[:], in_=calc_)
                # ceq0 = 1 - sgn^2 ; ceq1 = relu(1 - |calc - 1|)   (exact {0,1})
                nc.scalar.activation(out=s2_[:], in_=sgn[:], func=ACT.Square)
                nc.scalar.activation(out=a1_[:], in_=calc_, func=ACT.Abs,
                                     scale=-1.0, bias=1.0)
                nc.scalar.activation(out=ceq0[:], in_=s2_[:], func=ACT.Identity,
                                     scale=-1.0, bias=1.0)
                nc.scalar.activation(out=ceq1[:], in_=a1_[:], func=ACT.Relu,
                                     scale=-1.0, bias=1.0)
                # z (segment-major dense write; strided/broadcast reads)
                MT = M[:].rearrange("p k s -> p s k")
                sgnB = sgn[:].broadcast_to([P, f, 3]).rearrange("p k s -> p s k")
                if OPTS["z_arith"]:
                    # z = relu(1 - (M - sgn)^2), exact on {-2..2}
                    dz = wkB.tile([P, 3, f], BF16, tag="dz")
                    zsq = wkB.tile([P, 3, f], BF16, tag="zsq")
                    nc.gpsimd.tensor_tensor(out=dz[:], in0=MT, in1=sgnB,
                                            op=A.subtract)
                    yield
                    nc.scalar.activation(out=zsq[:], in_=dz[:], func=ACT.Square)
                    yield
                    nc.scalar.activation(out=z[:], in_=zsq[:], func=ACT.Relu,
                                         scale=-1.0, bias=1.0)
                else:
                    nc.vector.tensor_tensor(out=z[:], in0=MT, in1=sgnB,
                                            op=A.is_equal)
                d["z"], d["ceq0"], d["ceq1"] = z, ceq0, ceq1

            def stageB(t):
                d = st[t]
                z, ceq0, ceq1 = d["z"], d["ceq0"], d["ceq1"]
                SEL = wkA.tile([P, f, 3], FP32, tag="SEL")
                cmp = wkA.tile([P, 3, f], FP32, tag="cmp")
                sm = wkB.tile([P, 8, f], BF16, tag="smB")
                gun, gud, gnd, bu, bn, nbu, ngnd, bd = (
                    sm[:, i, :] for i in range(8)
                )
                wd = wkA.tile([P, f], BF16, tag="wd")
                wu8 = wkA.tile([P, f], U8, tag="wu8")
                wn8 = wkA.tile([P, f], U8, tag="wn8")

                nc.scalar.copy(out=SEL[:], in_=d["X2"])
                yield
                nc.vector.copy_predicated(
                    out=SEL[:], mask=ceq0[:].broadcast_to([P, f, 3]), data=d["X1"]
                )
                yield
                nc.vector.copy_predicated(
                    out=SEL[:], mask=ceq1[:].broadcast_to([P, f, 3]), data=d["X0"]
                )
                yield
                nc.vector.tensor_tensor(
                    out=cmp[:], in0=SEL[:].rearrange("p k s -> p s k"), in1=z[:],
                    op=A.mult)
                yield
                c_u, c_n, c_d = (cmp[:, s, :] for s in range(3))
                nc.vector.tensor_tensor(out=gun, in0=c_u, in1=c_n, op=A.is_ge)
                yield
                nc.vector.tensor_tensor(out=gud, in0=c_u, in1=c_d, op=A.is_ge)
                yield
                nc.vector.tensor_tensor(out=gnd, in0=c_n, in1=c_d, op=A.is_ge)
                yield
                nc.vector.tensor_tensor(out=bu, in0=gun, in1=gud, op=A.mult)
                yield
                nc.vector.tensor_tensor(out=bn, in0=gnd, in1=bu, op=A.is_gt)
                nc.vector.tensor_scalar(out=nbu, in0=bu, scalar1=-1.0,
                                        scalar2=1.0, op0=A.mult, op1=A.add)
                nc.vector.tensor_scalar(out=ngnd, in0=gnd, scalar1=-1.0,
                                        scalar2=1.0, op0=A.mult, op1=A.add)
                yield
                nc.vector.tensor_tensor(out=bd, in0=nbu, in1=ngnd, op=A.mult)
                z_u, z_n, z_d = (z[:, s, :] for s in range(3))
                nc.vector.tensor_tensor(out=wu8[:], in0=bu, in1=z_u, op=A.mult)
                yield
                nc.vector.tensor_tensor(out=wn8[:], in0=bn, in1=z_n, op=A.mult)
                yield
                nc.vector.tensor_tensor(out=wd[:], in0=bd, in1=z_d, op=A.mult)
                d["wd"], d["wu8"], d["wn8"] = wd, wu8, wn8

            def stageC(t):
                d = st.pop(t)
                ot = io2.tile([P, f * 3], FP32, tag="ot")
                O3 = ot[:].rearrange("p (k c) -> p k c", c=3)
                nc.vector.tensor_tensor(
                    out=O3, in0=d["D"], in1=d["wd"][:].broadcast_to([P, f, 3]),
                    op=A.mult,
                )
                yield
                nc.vector.copy_predicated(
                    out=O3, mask=d["wn8"][:].broadcast_to([P, f, 3]), data=d["Nv"]
                )
                yield
                nc.vector.copy_predicated(
                    out=O3, mask=d["wu8"][:].broadcast_to([P, f, 3]), data=d["U"]
                )
                r0, r1 = t * P * f, (t + 1) * P * f
                nc.sync.dma_start(
                    out=y[r0:r1, :].rearrange("(p f) c -> p (f c)", p=P),
                    in_=ot[:],
                )

            def drain(gens):
                gens = [g for g in gens if g is not None]
                while gens:
                    nxt = []
                    for g in gens:
                        try:
                            next(g)
                            nxt.append(g)
                        except StopIteration:
                            pass
                    gens = nxt

            def chain(t):
                yield from stageB(t)
                yield from stageC(t)

            if skew:
                for i in range(ntiles + 2):
                    if i < ntiles:
                        dma_in(i)
                    drain([
                        stageA(i - 1) if 0 <= i - 1 < ntiles else None,
                        chain(i - 2) if 0 <= i - 2 < ntiles else None,
                    ])
            else:
                for t in range(ntiles):
                    dma_in(t)
                    drain([stageA(t)])
                    drain([stageB(t)])
                    drain([stageC(t)])

    return nc


_CACHED = {}


def _get_kernel(rows_per_core: int, f: int) -> bass.Bass:
    key = (rows_per_core, f)
    if key not in _CACHED:
        nc = build_kernel_v4(rows_per_core, f)
        nc.finalize()
        legalize_multi_waits(nc)
        _CACHED[key] = nc
    return _CACHED[key]


LAST_RES = None  # test-harness hook: BassKernelResults of the last run


def kernel(x: np.ndarray) -> np.ndarray:
    global LAST_RES
    x = np.ascontiguousarray(np.asarray(x), dtype=np.float32)
    n = x.shape[0]
    assert n % N_CORES == 0
    rpc = n // N_CORES
    f = 512
    nc = _get_kernel(rpc, f)
    shards = [x[i * rpc:(i + 1) * rpc] for i in range(N_CORES)]
    in_maps = [{"x": s} for s in shards]
    LAST_RES = run_bass_kernel_spmd(nc, in_maps, list(range(N_CORES)))
    res = LAST_RES.results
    return np.concatenate([r["y"] for r in res], axis=0)

